# revision 1
# baseline (speedup 1.0000x reference)
"""PhysicsGAT (2x GATv2 + pooling) on 8 trn2 cores.

- Nodes partitioned by graph (8 graphs/core), G_PAD-aligned slots.
- Self-loops handled densely (no gather); random edges dst-sharded.
- conv1 per 128-edge block: xl_src from f16 dma_gather (transpose mode,
  per-chunk compact int16 tables); xr_dst from dma_gather on local DRAM
  table; z accumulated in PSUM (identity-matmul add); attention on
  DVE/ACT; scatter via one-hot matmul with fused denominator column.
- conv2: xl2 all-gathered f32 via 8 chunked collectives overlapping
  conv1; per-edge gathers split into two table halves (int16 limit);
  xr2 from local table.
- Pooling: sum via one-hot matmul accumulated across tiles; max via PE
  transpose + running max; final linear on-chip.
"""
import os
import numpy as np
import concourse.bass as bass
import concourse.bacc as bacc
import concourse.mybir as mybir
import concourse.tile as tile
from concourse.bass_utils import run_bass_kernel_spmd
from concourse.masks import make_identity

NCORES = 8
B = 64
GPC = 8
IN = 5
H1, C1 = 4, 64
C2 = 64

f32 = mybir.dt.float32
f16 = mybir.dt.float16
i16 = mybir.dt.int16
EXP = mybir.ActivationFunctionType.Exp
COPY = mybir.ActivationFunctionType.Copy
AX = mybir.AxisListType.X
ADD = mybir.AluOpType.add
SUB = mybir.AluOpType.subtract
MAXO = mybir.AluOpType.max
MULT = mybir.AluOpType.mult
EQ = mybir.AluOpType.is_equal
BYP = mybir.AluOpType.bypass

CH1 = 32   # conv1 source-gather chunk, in blocks
CHX = 16   # all other gather chunks, in blocks

_cache = {}
_last = None


def _build(cfg):
    G_PAD, K1, K2A, K2B, UPAD = (cfg["G_PAD"], cfg["K1"], cfg["K2A"],
                                 cfg["K2B"], cfg["UPAD"])
    NLOC = GPC * G_PAD
    NT = NLOC // 128
    TPG = G_PAD // 128
    NB1 = int(sum(K1))
    NBA = int(sum(K2A))
    NBB = int(sum(K2B))
    NB2 = NBA + NBB
    HALF = 4 * NLOC
    UTOT = int(sum(UPAD))
    NCH1 = (NB1 + CH1 - 1) // CH1
    NCHX1 = (NB1 + CHX - 1) // CHX
    NCHA = max(1, (NBA + CHX - 1) // CHX)
    NCHB = max(1, (NBB + CHX - 1) // CHX)
    NCH2X = (NB2 + CHX - 1) // CHX
    UB = np.concatenate([[0], np.cumsum(UPAD)]).astype(int)
    BS1 = np.concatenate([[0], np.cumsum(K1)]).astype(int)
    BSA = np.concatenate([[0], np.cumsum(K2A)]).astype(int)
    BSB = np.concatenate([[0], np.cumsum(K2B)]).astype(int)
    S2B = [int(BSA[t] + BSB[t]) for t in range(NT + 1)]

    KTL = int(os.environ.get("KTL", "0"))
    nc = bacc.Bacc("TRN2", target_bir_lowering=False, debug=False,
                   num_devices=1 if KTL else NCORES)

    P = lambda n, s, d: nc.declare_dram_parameter(n, s, d, isOutput=False)
    xpT_d = P("xpT", [8, NLOC], f16)
    xtab = P("xtab", [UTOT, 128], f16)
    xidx = P("xidx", [128, NCH1 * CH1 * 8], i16)
    xr1i = P("xr1i", [128, NCHX1 * CHX * 8], i16)
    ds1_d = P("ds1", [128, NB1], f16)
    xleAi = P("xleAi", [128, NCHA * CHX * 8], i16)
    xleBi = P("xleBi", [128, NCHB * CHX * 8], i16)
    xr2i = P("xr2i", [128, NCH2X * CHX * 8], i16)
    ds2_d = P("ds2", [128, max(NB2, 1)], f16)
    wlr1_d = P("wlr1", [8, 512], f16)
    att1b_d = P("att1b", [128, 512], f16)
    bias1b_d = P("bias1b", [128, 256], f32)
    w2cat_d = P("w2cat", [128, 256], f16)    # [p, (slab, col)]
    blr2b_d = P("blr2b", [128, 128], f32)
    att2b_d = P("att2b", [128, 128], f16)
    bias2b_d = P("bias2b", [128, 64], f32)
    iota_d = P("iota16", [128, 128], f16)
    mk8_d = P("mk8", [128, NT * 8], f32)
    bgp_d = P("bgp", [128, NT], f32)
    rcntT_d = P("rcntT", [64, 8], f32)
    wlin_d = P("wlin", [64, 6], f32)         # [p, (slab, col)]
    blinb_d = P("blinb", [GPC, 2], f32)
    pooled = nc.declare_dram_parameter("pooled", [GPC, 2], f32,
                                       isOutput=True)

    xr1tab = nc.dram_tensor("xr1tab", [NLOC, 256], f16, kind="Internal")
    xr2tab = nc.dram_tensor("xr2tab", [NLOC, 64], f32, kind="Internal")
    ag_out = nc.dram_tensor("ag_out", [NCORES * NLOC, 64], f32,
                            kind="Internal")

    with tile.TileContext(nc) as tc:
        with (
            tc.tile_pool(name="const", bufs=1) as cp,
            tc.tile_pool(name="wk", bufs=4) as pool,
            tc.tile_pool(name="ep", bufs=3) as epool,
            tc.tile_pool(name="gx", bufs=2) as gxp,
            tc.tile_pool(name="gr", bufs=2) as grp_,
            tc.tile_pool(name="ga", bufs=2) as gap,
            tc.tile_pool(name="gb", bufs=2) as gbp,
            tc.tile_pool(name="g2", bufs=2) as g2p,
            tc.tile_pool(name="psz", bufs=3, space="PSUM") as psZ,
            tc.tile_pool(name="psa", bufs=2, space="PSUM") as psA,
            tc.tile_pool(name="pst", bufs=1, space="PSUM") as psT,
            tc.tile_pool(name="psl", bufs=1, space="PSUM") as psL,
            tc.tile_pool(name="psp", bufs=1, space="PSUM") as psP,
            tc.tile_pool(name="dio", bufs=1, space="DRAM") as dpool,
        ):
            # ---- constants ----
            def _ld(name, shape, dt, src):
                t = cp.tile(shape, dt, name=name)
                nc.sync.dma_start(out=t[:], in_=src[:])
                return t
            wlr1 = _ld("wlr1", [8, 512], f16, wlr1_d)
            att1b = _ld("att1b", [128, 512], f16, att1b_d)
            bias1b = _ld("bias1b", [128, 256], f32, bias1b_d)
            w2cat = _ld("w2cat", [128, 256], f16, w2cat_d)
            blr2b = _ld("blr2b", [128, 128], f32, blr2b_d)
            att2b = _ld("att2b", [128, 128], f16, att2b_d)
            bias2b = _ld("bias2b", [128, 64], f32, bias2b_d)
            iota16 = _ld("iota16", [128, 128], f16, iota_d)
            mk8 = _ld("mk8", [128, NT * 8], f32, mk8_d)
            bgp = _ld("bgp", [128, NT], f32, bgp_d)
            rcntT = _ld("rcntT", [64, 8], f32, rcntT_d)
            wlin = _ld("wlin", [64, 6], f32, wlin_d)
            blinb = _ld("blinb", [GPC, 2], f32, blinb_d)
            ds1 = _ld("ds1", [128, NB1], f16, ds1_d)
            ds2 = _ld("ds2", [128, max(NB2, 1)], f16, ds2_d)
            xidx_sb = _ld("xidx_sb", [128, NCH1 * CH1 * 8], i16, xidx)
            xr1i_sb = _ld("xr1i_sb", [128, NCHX1 * CHX * 8], i16, xr1i)
            xleAi_sb = _ld("xleAi_sb", [128, NCHA * CHX * 8], i16, xleAi)
            xleBi_sb = _ld("xleBi_sb", [128, NCHB * CHX * 8], i16, xleBi)
            xr2i_sb = _ld("xr2i_sb", [128, NCH2X * CHX * 8], i16, xr2i)
            id16 = cp.tile([128, 128], f16)
            make_identity(nc, id16[:])
            wS1 = cp.tile([128, NT * 4], f32)
            rhsS1 = cp.tile([128, NT * 256], f16)
            wS2 = cp.tile([128, NT], f32)
            rhsS2 = cp.tile([128, NT * 64], f16)
            mxAcc = cp.tile([64, GPC], f32)
            nbias = cp.tile([128, 1], f32)
            nc.vector.memset(nbias[:], -4.0)
            ag_in1 = dpool.tile([NLOC, 64], f32, name="agin")
            psPool = psP.tile([64, GPC], f32, tag="pool")

            # ============ phase 0: local transforms + conv1 self ==========
            for t in range(NT):
                xpT_t = pool.tile([8, 128], f16, tag="xpT")
                nc.sync.dma_start(out=xpT_t[:],
                                  in_=xpT_d[:, t * 128:(t + 1) * 128])
                ps0 = psZ.tile([128, 512], f32, tag="z")
                nc.tensor.matmul(ps0[:], xpT_t[:], wlr1[:],
                                 start=True, stop=True)
                xr1sb = pool.tile([128, 256], f16, tag="xr1sb")
                nc.scalar.activation(xr1sb[:], ps0[:, 256:512], COPY)
                nc.sync.dma_start(out=xr1tab[t * 128:(t + 1) * 128, :],
                                  in_=xr1sb[:])
                zs = pool.tile([128, 256], f16, tag="zs")
                nc.vector.tensor_tensor(out=zs[:], in0=ps0[:, 0:256],
                                        in1=xr1sb[:], op=ADD)
                zsl = pool.tile([128, 256], f16, tag="zsl")
                nc.vector.scalar_tensor_tensor(
                    out=zsl[:], in0=zs[:], scalar=0.2, in1=zs[:],
                    op0=MULT, op1=MAXO)
                zsa = pool.tile([128, 256], f16, tag="zsa")
                nc.vector.tensor_mul(zsa[:], zsl[:], att1b[:, 0:256])
                lgS = pool.tile([128, 4], f32, tag="lgS")
                nc.vector.tensor_reduce(
                    lgS[:], zsa[:].rearrange("p (h c) -> p h c", c=C1),
                    AX, ADD)
                nc.scalar.activation(wS1[:, t * 4:(t + 1) * 4], lgS[:], EXP,
                                     bias=nbias[:, 0:1])
                nc.vector.tensor_mul(
                    rhsS1[:, t * 256:(t + 1) * 256].rearrange(
                        "p (h c) -> p h c", c=C1),
                    ps0[:, 0:256].rearrange("p (h c) -> p h c", c=C1),
                    wS1[:, t * 4:(t + 1) * 4].unsqueeze(2).to_broadcast(
                        (128, H1, C1)))

            KPH = int(os.environ.get("KPHASE", "4"))
            if KPH == 5:
                # xs gathers only
                for _ch in range((NB1 + CH1 - 1) // CH1):
                    _nblk = min(CH1, NB1 - _ch * CH1)
                    _tl = gxp.tile([128, CH1 * 128], f16, tag="xs")
                    nc.gpsimd.dma_gather(
                        out_ap=_tl[:, 0:_nblk * 128].rearrange(
                            "p (a b) -> p a b", a=1),
                        in_ap=xtab[int(UB[_ch]):int(UB[_ch + 1]), :],
                        idxs_ap=xidx_sb[:, _ch * CH1 * 8:_ch * CH1 * 8 + _nblk * 8],
                        num_idxs=_nblk * 128, num_idxs_reg=_nblk * 128,
                        elem_size=128, transpose=True,
                        single_packet=False)
            if KPH == 6:
                # xr gathers only
                for _ch in range((NB1 + CHX - 1) // CHX):
                    _nblk = min(CHX, NB1 - _ch * CHX)
                    _tl = grp_.tile([128, CHX * 256], f16, tag="xr")
                    nc.gpsimd.dma_gather(
                        out_ap=_tl[:, 0:_nblk * 256].rearrange(
                            "p (a b) -> p a b", b=256),
                        in_ap=xr1tab[:],
                        idxs_ap=xr1i_sb[:, _ch * CHX * 8:_ch * CHX * 8 + _nblk * 8],
                        num_idxs=_nblk * 128, num_idxs_reg=_nblk * 128,
                        elem_size=256, transpose=False,
                        single_packet=False)
            if KPH >= 1 and KPH <= 4:
                # ============ phase 1: conv1 edge blocks + epilogue =========
              xs_t = {}
              xr_t = {}

              def _xs(j):
                  ch = j // CH1
                  if ch not in xs_t:
                      nblk = min(CH1, NB1 - ch * CH1)
                      tl = gxp.tile([128, CH1 * 128], f16, tag="xs")
                      nc.gpsimd.dma_gather(
                          out_ap=tl[:, 0:nblk * 128].rearrange(
                              "p (a b) -> p a b", a=1),
                          in_ap=xtab[int(UB[ch]):int(UB[ch + 1]), :],
                          idxs_ap=xidx_sb[:, ch * CH1 * 8:ch * CH1 * 8 + nblk * 8],
                          num_idxs=nblk * 128, num_idxs_reg=nblk * 128,
                          elem_size=128, transpose=True,
                        single_packet=False)
                      xs_t[ch] = tl
                  return xs_t[ch], (j % CH1) * 128

              def _xr(j):
                  ch = j // CHX
                  if ch not in xr_t:
                      nblk = min(CHX, NB1 - ch * CHX)
                      tl = grp_.tile([128, CHX * 256], f16, tag="xr")
                      nc.gpsimd.dma_gather(
                          out_ap=tl[:, 0:nblk * 256].rearrange(
                              "p (a b) -> p a b", b=256),
                          in_ap=xr1tab[:],
                          idxs_ap=xr1i_sb[:, ch * CHX * 8:ch * CHX * 8 + nblk * 8],
                          num_idxs=nblk * 128, num_idxs_reg=nblk * 128,
                          elem_size=256, transpose=False,
                        single_packet=False)
                      xr_t[ch] = tl
                  return xr_t[ch], (j % CHX) * 256

              for t in range(NT):
                  k1 = int(K1[t])
                  aggP = None
                  if k1 > 0:
                      aggP = psA.tile([128, 260], f32, tag="agg")
                  for k0 in range(0, k1, 2):
                      ks = list(range(k0, min(k0 + 2, k1)))
                      js = [int(BS1[t]) + k for k in ks]
                      ng = len(js)
                      W = ng * 256
                      xrefs = [_xs(j) for j in js]
                      rrefs = [_xr(j) for j in js]
                      psz = psZ.tile([128, 512], f32, tag="z")
                      for i in range(ng):
                          xt, xc = xrefs[i]
                          rt, rc = rrefs[i]
                          nc.tensor.matmul(
                              psz[:, i * 256:(i + 1) * 256],
                              xt[0:8, xc:xc + 128], wlr1[:, 0:256],
                              start=True, stop=False)
                          nc.tensor.matmul(
                              psz[:, i * 256:(i + 1) * 256], id16[:],
                              rt[:, rc:rc + 256], start=False, stop=True)
                      oh = pool.tile([128, 256], f16, tag="oh")
                      nc.vector.tensor_tensor(
                          out=oh[:, 0:ng * 128].rearrange(
                              "p (g b) -> p g b", b=128),
                          in0=ds1[:, js[0]:js[0] + ng].unsqueeze(2)
                          .to_broadcast((128, ng, 128)),
                          in1=iota16[:].unsqueeze(1).to_broadcast(
                              (128, ng, 128)),
                          op=EQ)
                      zq = pool.tile([128, 512], f16, tag="zq")
                      nc.scalar.activation(zq[:, 0:W], psz[:, 0:W], COPY,
                                           scale=0.2)
                      zl = pool.tile([128, 512], f16, tag="zl")
                      nc.vector.tensor_tensor(out=zl[:, 0:W], in0=psz[:, 0:W],
                                              in1=zq[:, 0:W], op=MAXO)
                      za = pool.tile([128, 512], f16, tag="za")
                      nc.vector.tensor_mul(za[:, 0:W], zl[:, 0:W],
                                           att1b[:, 0:W])
                      lg = pool.tile([128, 8], f32, tag="lg")
                      nc.vector.tensor_reduce(
                          lg[:, 0:ng * 4],
                          za[:, 0:W].rearrange("p (gh c) -> p gh c", c=C1),
                          AX, ADD)
                      wv = pool.tile([128, 8], f32, tag="wv")
                      nc.scalar.activation(wv[:, 0:ng * 4], lg[:, 0:ng * 4],
                                           EXP, bias=nbias[:, 0:1])
                      rhs = pool.tile([128, 520], f16, tag="rhs")
                      rga = rhs[:, 0:ng * 260].rearrange(
                          "p (g x) -> p g x", x=260)
                      nc.scalar.activation(
                          rga[:, :, 256:260],
                          wv[:, 0:ng * 4].rearrange("p (g x) -> p g x", x=4),
                          COPY)
                      xlv = pool.tile([128, 512], f16, tag="xlv")
                      same_chunk = (ng == 2 and rrefs[0][0] is rrefs[1][0]
                                    and rrefs[0][1] + 256 == rrefs[1][1])
                      if ng == 1 or same_chunk:
                          rt0, rc0 = rrefs[0]
                          nc.vector.tensor_tensor(
                              out=xlv[:, 0:W], in0=psz[:, 0:W],
                              in1=rt0[:, rc0:rc0 + W], op=SUB)
                      else:
                          for i in range(ng):
                              rt, rc = rrefs[i]
                              nc.vector.tensor_tensor(
                                  out=xlv[:, i * 256:(i + 1) * 256],
                                  in0=psz[:, i * 256:(i + 1) * 256],
                                  in1=rt[:, rc:rc + 256], op=SUB)
                      for i in range(ng):
                          nc.vector.tensor_mul(
                              rhs[:, i * 260:i * 260 + 256].rearrange(
                                  "p (h c) -> p h c", c=C1),
                              xlv[:, i * 256:(i + 1) * 256].rearrange(
                                  "p (h c) -> p h c", c=C1),
                              wv[:, i * 4:(i + 1) * 4].unsqueeze(2)
                              .to_broadcast((128, H1, C1)))
                      for i, k in enumerate(ks):
                          nc.tensor.matmul(
                              aggP[:], oh[:, i * 128:(i + 1) * 128],
                              rhs[:, i * 260:(i + 1) * 260],
                              start=(k == 0), stop=(k == k1 - 1))

                  # ---- conv1 epilogue, tile t ----
                  den = epool.tile([128, 4], f32, tag="den")
                  num = epool.tile([128, 256], f32, tag="num")
                  if k1 > 0:
                      nc.vector.tensor_add(den[:], aggP[:, 256:260],
                                           wS1[:, t * 4:(t + 1) * 4])
                      nc.vector.tensor_add(num[:], aggP[:, 0:256],
                                           rhsS1[:, t * 256:(t + 1) * 256])
                  else:
                      nc.vector.tensor_copy(den[:], wS1[:, t * 4:(t + 1) * 4])
                      nc.vector.tensor_copy(num[:],
                                            rhsS1[:, t * 256:(t + 1) * 256])
                  rr = epool.tile([128, 4], f32, tag="rr")
                  nc.vector.reciprocal(rr[:], den[:])
                  h1 = epool.tile([128, 256], f32, tag="h1")
                  nc.vector.tensor_mul(
                      h1[:].rearrange("p (h c) -> p h c", c=C1),
                      num[:].rearrange("p (h c) -> p h c", c=C1),
                      rr[:].unsqueeze(2).to_broadcast((128, H1, C1)))
                  nc.vector.tensor_add(h1[:], h1[:], bias1b[:])
                  nmin = epool.tile([128, 256], f32, tag="nmin")
                  nc.vector.tensor_scalar_min(nmin[:], h1[:], 0.0)
                  eex = epool.tile([128, 256], f32, tag="eex")
                  nc.scalar.activation(eex[:], nmin[:], EXP)
                  x1 = epool.tile([128, 256], f16, tag="x1")
                  nc.vector.scalar_tensor_tensor(
                      out=x1[:], in0=h1[:], scalar=0.0, in1=eex[:],
                      op0=MAXO, op1=ADD)
                  x1T = epool.tile([128, 256], f16, tag="x1T")
                  for sl in range(2):
                      tp = psT.tile([128, 128], f16, tag="tp")
                      nc.tensor.transpose(
                          tp[:], x1[:, sl * 128:(sl + 1) * 128], id16[:])
                      nc.scalar.activation(
                          x1T[:, sl * 128:(sl + 1) * 128], tp[:], COPY)
                  psl = psL.tile([128, 128], f32, tag="L")
                  nc.tensor.matmul(psl[:], x1T[:, 0:128], w2cat[:, 0:128],
                                   start=True, stop=False)
                  nc.tensor.matmul(psl[:], x1T[:, 128:256], w2cat[:, 128:256],
                                   start=False, stop=True)
                  xlr2 = epool.tile([128, 128], f32, tag="xlr2")
                  nc.vector.tensor_add(xlr2[:], psl[:], blr2b[:])
                  s = t // TPG
                  r0 = s * G_PAD + (t % TPG) * 128
                  nc.sync.dma_start(out=ag_in1[r0:r0 + 128, :],
                                    in_=xlr2[:, 0:64])
                  nc.sync.dma_start(out=xr2tab[t * 128:(t + 1) * 128, :],
                                    in_=xlr2[:, 64:128])
                  zs2 = epool.tile([128, 64], f16, tag="zs2")
                  nc.vector.tensor_tensor(out=zs2[:], in0=xlr2[:, 0:64],
                                          in1=xlr2[:, 64:128], op=ADD)
                  zs2l = epool.tile([128, 64], f16, tag="zs2l")
                  nc.vector.scalar_tensor_tensor(
                      out=zs2l[:], in0=zs2[:], scalar=0.2, in1=zs2[:],
                      op0=MULT, op1=MAXO)
                  zs2a = epool.tile([128, 64], f16, tag="zs2a")
                  nc.vector.tensor_mul(zs2a[:], zs2l[:], att2b[:, 0:64])
                  lg2S = epool.tile([128, 1], f32, tag="lg2S")
                  nc.vector.tensor_reduce(lg2S[:], zs2a[:], AX, ADD)
                  nc.scalar.activation(wS2[:, t:t + 1], lg2S[:], EXP,
                                       bias=nbias[:, 0:1])
                  nc.scalar.activation(rhsS2[:, t * 64:(t + 1) * 64],
                                       xlr2[:, 0:64], COPY,
                                       scale=wS2[:, t:t + 1])


            if KPH >= 2:
                if KTL:
                    for _c in range(NCORES):
                        nc.sync.dma_start(
                            out=ag_out[_c * NLOC:(_c + 1) * NLOC, :],
                            in_=ag_in1[:])
                else:
                    nc.gpsimd.collective_compute(
                        "AllGather", BYP,
                        replica_groups=[list(range(NCORES))],
                        ins=[ag_in1[:].opt()], outs=[ag_out[:].opt()])

            if KPH >= 3:
              # ============ phase 2: conv2 ==================================
              xa_t = {}
              xb_t = {}
              x2r_t = {}

              def _mk_fetch(cache, pool_, idx_sb, src_ap, nbtot):
                  def f(j):
                      ch = j // CHX
                      if ch not in cache:
                          nblk = min(CHX, nbtot - ch * CHX)
                          tl = pool_.tile([128, CHX * 64], f32, tag="g")
                          nc.gpsimd.dma_gather(
                              out_ap=tl[:, 0:nblk * 64].rearrange(
                                  "p (a b) -> p a b", b=64),
                              in_ap=src_ap,
                              idxs_ap=idx_sb[:, ch * CHX * 8:ch * CHX * 8 + nblk * 8],
                              num_idxs=nblk * 128, num_idxs_reg=nblk * 128,
                              elem_size=64, transpose=False,
                            single_packet=False)
                          cache[ch] = tl
                      return cache[ch], (j % CHX) * 64
                  return f

              _xa = _mk_fetch(xa_t, gap, xleAi_sb, ag_out[0:HALF, :], NBA)
              _xb = _mk_fetch(xb_t, gbp, xleBi_sb, ag_out[HALF:2 * HALF, :],
                              NBB)
              _x2r = _mk_fetch(x2r_t, g2p, xr2i_sb, xr2tab[:], NB2)

              for t in range(NT):
                  kA, kB = int(K2A[t]), int(K2B[t])
                  ktot = kA + kB
                  agg2P = None
                  if ktot > 0:
                      agg2F = psA.tile([128, 260], f32, tag="agg")
                      agg2P = agg2F[:, 0:65]
                  for half, (kcnt, base, fetch, koff) in enumerate((
                          (kA, int(BSA[t]), _xa, 0),
                          (kB, int(BSB[t]), _xb, kA))):
                      for k0 in range(0, kcnt, 2):
                          ks = list(range(k0, min(k0 + 2, kcnt)))
                          ng = len(ks)
                          W2 = ng * 64
                          js = [base + k for k in ks]
                          s2js = [S2B[t] + koff + k for k in ks]
                          erefs = [fetch(j) for j in js]
                          rrefs = [_x2r(sj) for sj in s2js]
                          oh2 = pool.tile([128, 256], f16, tag="oh2")
                          nc.vector.tensor_tensor(
                              out=oh2[:, 0:ng * 128].rearrange(
                                  "p (g b) -> p g b", b=128),
                              in0=ds2[:, s2js[0]:s2js[0] + ng].unsqueeze(2)
                              .to_broadcast((128, ng, 128)),
                              in1=iota16[:].unsqueeze(1).to_broadcast(
                                  (128, ng, 128)),
                              op=EQ)
                          z2 = pool.tile([128, 128], f16, tag="z2")
                          e_same = (ng == 2 and erefs[0][0] is erefs[1][0]
                                    and erefs[0][1] + 64 == erefs[1][1])
                          r_same = (ng == 2 and rrefs[0][0] is rrefs[1][0]
                                    and rrefs[0][1] + 64 == rrefs[1][1])
                          if ng == 1 or (e_same and r_same):
                              et, ec = erefs[0]
                              rt, rc = rrefs[0]
                              nc.vector.tensor_tensor(
                                  out=z2[:, 0:W2], in0=et[:, ec:ec + W2],
                                  in1=rt[:, rc:rc + W2], op=ADD)
                          else:
                              for i in range(ng):
                                  et, ec = erefs[i]
                                  rt, rc = rrefs[i]
                                  nc.vector.tensor_tensor(
                                      out=z2[:, i * 64:(i + 1) * 64],
                                      in0=et[:, ec:ec + 64],
                                      in1=rt[:, rc:rc + 64], op=ADD)
                          z2l = pool.tile([128, 128], f16, tag="z2l")
                          nc.vector.scalar_tensor_tensor(
                              out=z2l[:, 0:W2], in0=z2[:, 0:W2], scalar=0.2,
                              in1=z2[:, 0:W2], op0=MULT, op1=MAXO)
                          z2a = pool.tile([128, 128], f16, tag="z2a")
                          nc.vector.tensor_mul(z2a[:, 0:W2], z2l[:, 0:W2],
                                               att2b[:, 0:W2])
                          lg2 = pool.tile([128, 2], f32, tag="lg2")
                          nc.vector.tensor_reduce(
                              lg2[:, 0:ng],
                              z2a[:, 0:W2].rearrange("p (g c) -> p g c", c=64),
                              AX, ADD)
                          w2 = pool.tile([128, 2], f32, tag="w2")
                          nc.scalar.activation(w2[:, 0:ng], lg2[:, 0:ng], EXP,
                                           bias=nbias[:, 0:1])
                          rhs2 = pool.tile([128, 130], f16, tag="rhs2")
                          rg2 = rhs2[:, 0:ng * 65].rearrange(
                              "p (g x) -> p g x", x=65)
                          nc.scalar.activation(
                              rg2[:, :, 64:65],
                              w2[:, 0:ng].unsqueeze(2), COPY)
                          for i in range(ng):
                              et, ec = erefs[i]
                              nc.scalar.activation(
                                  rhs2[:, i * 65:i * 65 + 64],
                                  et[:, ec:ec + 64],
                                  COPY, scale=w2[:, i:i + 1])
                          for i, k in enumerate(ks):
                              kk = koff + k
                              nc.tensor.matmul(
                                  agg2F[:, 0:65], oh2[:, i * 128:(i + 1) * 128],
                                  rhs2[:, i * 65:(i + 1) * 65],
                                  start=(kk == 0), stop=(kk == ktot - 1))

                  # ---- conv2 epilogue + pooling, tile t ----
                  den2 = epool.tile([128, 1], f32, tag="den2")
                  num2 = epool.tile([128, 64], f32, tag="num2")
                  if ktot > 0:
                      nc.vector.tensor_add(den2[:], agg2F[:, 64:65],
                                           wS2[:, t:t + 1])
                      nc.vector.tensor_add(num2[:], agg2F[:, 0:64],
                                           rhsS2[:, t * 64:(t + 1) * 64])
                  else:
                      nc.vector.tensor_copy(den2[:], wS2[:, t:t + 1])
                      nc.vector.tensor_copy(num2[:],
                                            rhsS2[:, t * 64:(t + 1) * 64])
                  rr2 = epool.tile([128, 1], f32, tag="rr2")
                  nc.vector.reciprocal(rr2[:], den2[:])
                  h2 = epool.tile([128, 64], f32, tag="h2")
                  nc.vector.tensor_mul(h2[:], num2[:],
                                       rr2[:].to_broadcast((128, 64)))
                  nc.vector.tensor_add(h2[:], h2[:], bias2b[:])
                  nm2 = epool.tile([128, 64], f32, tag="nm2")
                  nc.vector.tensor_scalar_min(nm2[:], h2[:], 0.0)
                  ee2 = epool.tile([128, 64], f32, tag="ee2")
                  nc.scalar.activation(ee2[:], nm2[:], EXP)
                  x2 = epool.tile([128, 64], f32, tag="x2")
                  nc.vector.scalar_tensor_tensor(
                      out=x2[:], in0=h2[:], scalar=0.0, in1=ee2[:],
                      op0=MAXO, op1=ADD)
                  nc.tensor.matmul(psPool[:], x2[:], mk8[:, t * 8:(t + 1) * 8],
                                   start=(t == 0), stop=(t == NT - 1))
                  x2m = epool.tile([128, 64], f16, tag="x2m")
                  nc.vector.tensor_tensor(
                      out=x2m[:], in0=x2[:],
                      in1=bgp[:, t:t + 1].to_broadcast((128, 64)), op=ADD)
                  tpf = psT.tile([128, 256], f16, tag="tp")
                  nc.tensor.transpose(tpf[0:64, 0:128], x2m[:], id16[:])
                  mxv = epool.tile([64, 1], f32, tag="mxv")
                  nc.vector.tensor_reduce(mxv[:], tpf[0:64, 0:128], AX, MAXO)
                  g = t // TPG
                  if t % TPG == 0:
                      nc.vector.tensor_copy(mxAcc[:, g:g + 1], mxv[:])
                  else:
                      nc.vector.tensor_tensor(out=mxAcc[:, g:g + 1],
                                              in0=mxAcc[:, g:g + 1],
                                              in1=mxv[:], op=MAXO)

            # ============ phase 3: final linear ===========================
            outp = pool.tile([GPC, 2], f32, tag="outp")
            if KPH >= 3:
                smT = pool.tile([64, 8], f32, tag="smT")
                nc.scalar.activation(smT[:], psPool[:], COPY)
                meanT = pool.tile([64, 8], f32, tag="meanT")
                nc.vector.tensor_mul(meanT[:], smT[:], rcntT[:])
                finF = psL.tile([128, 128], f32, tag="L")
                finP = finF[0:GPC, 0:2]
                nc.tensor.matmul(finP, meanT[:], wlin[:, 0:2],
                                 start=True, stop=False)
                nc.tensor.matmul(finP, mxAcc[:], wlin[:, 2:4],
                                 start=False, stop=False)
                nc.tensor.matmul(finP, smT[:], wlin[:, 4:6],
                                 start=False, stop=True)
                nc.vector.tensor_add(outp[:], finP, blinb[:])
            else:
                nc.vector.memset(outp[:], 0.0)
            nc.sync.dma_start(out=pooled[:], in_=outp[:])

    nc.compile()
    return nc


def _wrap_idx(idx, ncols):
    """dma_gather idx layout: idx i at [i%16, i//16], replicated over the
    eight 16-partition groups; returns [128, ncols] int16."""
    out = np.zeros((128, ncols), np.int16)
    n = len(idx)
    cols = (n + 15) // 16
    flat = np.zeros(cols * 16, np.int64)
    flat[:n] = idx
    buf = flat.reshape(cols, 16).T.astype(np.int16)
    for g in range(8):
        out[g * 16:(g + 1) * 16, :cols] = buf
    return out


def _prep(inputs):
    x = np.asarray(inputs["x"], np.float32)
    ei = np.asarray(inputs["edge_index"]).astype(np.int64)
    bt = np.asarray(inputs["batch"]).astype(np.int64)
    N = x.shape[0]
    E = ei.shape[1]

    sizes = np.bincount(bt, minlength=B).astype(np.int64)
    start = np.zeros(B, np.int64)
    start[1:] = np.cumsum(sizes)[:-1]
    G_PAD = int(np.ceil(max(int(sizes.max()), 1) / 128.0)) * 128
    NLOC = GPC * G_PAD
    NT = NLOC // 128
    TPG = G_PAD // 128
    HALF = 4 * NLOC
    assert HALF <= 32768, f"half table too big for int16: {HALF}"

    rank = np.arange(N, dtype=np.int64) - start[bt]
    core_n = bt // GPC
    slot = bt % GPC
    loc = slot * G_PAD + rank
    # all-gather row id: [core, slot, rank]
    srow = core_n * NLOC + loc

    src, dst = ei[0], ei[1]
    ec = core_n[dst]
    dl = loc[dst]
    dtile = dl // 128
    dslot = dl % 128

    # ---------- conv1 layout ----------
    cnt1 = np.zeros((NCORES, NT), np.int64)
    for c in range(NCORES):
        cnt1[c] = np.bincount(dtile[ec == c], minlength=NT)
    K1 = np.ceil(cnt1.max(0) / 128.0).astype(np.int64)
    BS1 = np.concatenate([[0], np.cumsum(K1)]).astype(np.int64)
    NB1 = int(BS1[-1])
    E1 = NB1 * 128

    src1 = np.zeros((NCORES, E1), np.int64)          # global src node
    dst1 = np.zeros((NCORES, E1), np.int64)          # local dst node
    ds1v = np.full((NCORES, E1), -1.0, np.float32)   # dst slot (or -1)
    for c in range(NCORES):
        m = np.nonzero(ec == c)[0]
        order = np.argsort(dl[m], kind="stable")
        me = m[order]
        dt_ = dtile[me]
        gs = np.searchsorted(dt_, np.arange(NT), side="left")
        rpos = np.arange(len(me), dtype=np.int64) - gs[dt_]
        sl = BS1[dt_] * 128 + rpos
        src1[c, sl] = src[me]
        dst1[c, sl] = dl[me]
        ds1v[c, sl] = dslot[me]

    # conv1 src chunks: compact tables
    NCH1 = (NB1 + CH1 - 1) // CH1
    uniq_list = []   # [chunk][core] -> (uniq_ids, inv)
    UPAD = np.zeros(NCH1, np.int64)
    for ch in range(NCH1):
        s0, s1 = ch * CH1 * 128, min((ch + 1) * CH1, NB1) * 128
        per_core = []
        for c in range(NCORES):
            u, inv = np.unique(src1[c, s0:s1], return_inverse=True)
            per_core.append((u, inv))
        uniq_list.append(per_core)
        UPAD[ch] = int(np.ceil(max(len(u) for u, _ in per_core) / 16.0)) * 16
    UB = np.concatenate([[0], np.cumsum(UPAD)]).astype(np.int64)
    UTOT = int(UB[-1])

    xp16 = np.zeros((N, 128), np.float16)
    xp16[:, :IN] = x.astype(np.float16)
    xp16[:, IN] = 1.0

    xtab_np = np.zeros((NCORES, UTOT, 128), np.float16)
    xidx_np = np.zeros((NCORES, 128, NCH1 * CH1 * 8), np.int16)
    for ch in range(NCH1):
        nblk = min(CH1, NB1 - ch * CH1)
        for c in range(NCORES):
            u, inv = uniq_list[ch][c]
            xtab_np[c, UB[ch]:UB[ch] + len(u)] = xp16[u]
            xidx_np[c, :, ch * CH1 * 8:ch * CH1 * 8 + nblk * 8] = \
                _wrap_idx(inv, nblk * 8)

    # conv1 xr idx (local dst node ids)
    NCHX1 = (NB1 + CHX - 1) // CHX
    xr1i_np = np.zeros((NCORES, 128, NCHX1 * CHX * 8), np.int16)
    for ch in range(NCHX1):
        s0, s1 = ch * CHX * 128, min((ch + 1) * CHX, NB1) * 128
        nblk = (s1 - s0) // 128
        for c in range(NCORES):
            xr1i_np[c, :, ch * CHX * 8:ch * CHX * 8 + nblk * 8] = \
                _wrap_idx(dst1[c, s0:s1], nblk * 8)

    ds1_np = np.zeros((NCORES, 128, NB1), np.float16)
    for c in range(NCORES):
        ds1_np[c] = ds1v[c].reshape(NB1, 128).T.astype(np.float16)

    # ---------- conv2 layout (split by src half) ----------
    half_e = (srow[src] >= HALF).astype(np.int64)
    cntA = np.zeros((NCORES, NT), np.int64)
    cntB = np.zeros((NCORES, NT), np.int64)
    for c in range(NCORES):
        mc = ec == c
        cntA[c] = np.bincount(dtile[mc & (half_e == 0)], minlength=NT)
        cntB[c] = np.bincount(dtile[mc & (half_e == 1)], minlength=NT)
    K2A = np.ceil(cntA.max(0) / 128.0).astype(np.int64)
    K2B = np.ceil(cntB.max(0) / 128.0).astype(np.int64)
    BSA = np.concatenate([[0], np.cumsum(K2A)]).astype(np.int64)
    BSB = np.concatenate([[0], np.cumsum(K2B)]).astype(np.int64)
    NBA, NBB = int(BSA[-1]), int(BSB[-1])
    NB2 = NBA + NBB
    S2B = BSA + BSB   # per-tile S2 block base

    eA = np.zeros((NCORES, max(NBA, 1) * 128), np.int64)   # A src rows
    eB = np.zeros((NCORES, max(NBB, 1) * 128), np.int64)   # B src rows-HALF
    d2l = np.zeros((NCORES, max(NB2, 1) * 128), np.int64)  # local dst (S2)
    ds2v = np.full((NCORES, max(NB2, 1) * 128), -1.0, np.float32)
    for c in range(NCORES):
        for h in range(2):
            m = np.nonzero((ec == c) & (half_e == h))[0]
            order = np.argsort(dl[m], kind="stable")
            me = m[order]
            dt_ = dtile[me]
            gs = np.searchsorted(dt_, np.arange(NT), side="left")
            rpos = np.arange(len(me), dtype=np.int64) - gs[dt_]
            if h == 0:
                sl = BSA[dt_] * 128 + rpos
                eA[c, sl] = srow[src[me]]
                s2 = (S2B[dt_] + 0) * 128 + rpos
            else:
                sl = BSB[dt_] * 128 + rpos
                eB[c, sl] = srow[src[me]] - HALF
                s2 = (S2B[dt_] + K2A[dt_]) * 128 + rpos
            d2l[c, s2] = dl[me]
            ds2v[c, s2] = dslot[me]

    NCHA = max(1, (NBA + CHX - 1) // CHX)
    NCHB = max(1, (NBB + CHX - 1) // CHX)
    NCH2X = (NB2 + CHX - 1) // CHX
    xleAi_np = np.zeros((NCORES, 128, NCHA * CHX * 8), np.int16)
    xleBi_np = np.zeros((NCORES, 128, NCHB * CHX * 8), np.int16)
    xr2i_np = np.zeros((NCORES, 128, NCH2X * CHX * 8), np.int16)
    for c in range(NCORES):
        for ch in range(NCHA):
            s0 = ch * CHX * 128
            s1 = min((ch + 1) * CHX, NBA) * 128
            if s1 > s0:
                xleAi_np[c, :, ch * CHX * 8:ch * CHX * 8 + (s1 - s0) // 16] \
                    = _wrap_idx(eA[c, s0:s1], (s1 - s0) // 16)
        for ch in range(NCHB):
            s0 = ch * CHX * 128
            s1 = min((ch + 1) * CHX, NBB) * 128
            if s1 > s0:
                xleBi_np[c, :, ch * CHX * 8:ch * CHX * 8 + (s1 - s0) // 16] \
                    = _wrap_idx(eB[c, s0:s1], (s1 - s0) // 16)
        for ch in range(NCH2X):
            s0 = ch * CHX * 128
            s1 = min((ch + 1) * CHX, NB2) * 128
            xr2i_np[c, :, ch * CHX * 8:ch * CHX * 8 + (s1 - s0) // 16] = \
                _wrap_idx(d2l[c, s0:s1], (s1 - s0) // 16)

    ds2_np = np.zeros((NCORES, 128, max(NB2, 1)), np.float16)
    for c in range(NCORES):
        ds2_np[c] = ds2v[c].reshape(max(NB2, 1), 128).T.astype(np.float16)

    # ---------- weights / constants ----------
    gf = lambda k: np.asarray(inputs[k], np.float32)
    Wl1, bl1, Wr1, br1 = gf("Wl1"), gf("bl1"), gf("Wr1"), gf("br1")
    att1, bias1 = gf("att1"), gf("bias1")
    Wl2, bl2, Wr2, br2 = gf("Wl2"), gf("bl2"), gf("Wr2"), gf("br2")
    att2, bias2 = gf("att2"), gf("bias2")
    Wlin, blin = gf("Wlin"), gf("blin")

    wlr1_np = np.zeros((8, 512), np.float16)
    wlr1_np[:IN, 0:256] = Wl1.astype(np.float16)
    wlr1_np[IN, 0:256] = bl1.astype(np.float16)
    wlr1_np[:IN, 256:512] = Wr1.astype(np.float16)
    wlr1_np[IN, 256:512] = br1.astype(np.float16)
    att1b_np = np.tile(att1.reshape(1, 256), (128, 2)).astype(np.float16)
    bias1b_np = np.tile(bias1.reshape(1, 256), (128, 1))
    # conv2 weights with elu(-1) fold: x1Y = elu+1, so bias' = b - W.sum(0)
    w2cat_np = np.zeros((128, 256), np.float16)
    wcat = np.concatenate([Wl2, Wr2], axis=1)       # [256, 128]
    w2cat_np[:, 0:128] = wcat[0:128].astype(np.float16)
    w2cat_np[:, 128:256] = wcat[128:256].astype(np.float16)
    bl2f = bl2 - Wl2.sum(0)
    br2f = br2 - Wr2.sum(0)
    blr2b_np = np.tile(np.concatenate([bl2f, br2f]).reshape(1, 128),
                       (128, 1))
    att2b_np = np.tile(att2.reshape(1, 64), (128, 2)).astype(np.float16)
    bias2b_np = np.tile(bias2.reshape(1, 64), (128, 1))
    iota_np = np.tile(np.arange(128, dtype=np.float16).reshape(1, 128),
                      (128, 1))

    # pooling constants; x2Y = elu+1 shift folded into blinb
    mk8_np = np.zeros((NCORES, 128, NT * 8), np.float32)
    bgp_np = np.full((NCORES, 128, NT), -1e30, np.float32)
    rcntT_np = np.zeros((NCORES, 64, 8), np.float32)
    blinb_np = np.zeros((NCORES, GPC, 2), np.float32)
    Wm_s = Wlin[0:64].sum(0)
    Wx_s = Wlin[64:128].sum(0)
    Ws_s = Wlin[128:192].sum(0)
    for g in range(B):
        c, s = g // GPC, g % GPC
        n_g = int(sizes[g])
        for t in range(s * TPG, (s + 1) * TPG):
            base = t * 128 - s * G_PAD
            valid = np.clip(n_g - base, 0, 128)
            mk8_np[c, :int(valid), t * 8 + s] = 1.0
            bgp_np[c, :int(valid), t] = 0.0
        rcntT_np[c, :, s] = 1.0 / max(n_g, 1)
        blinb_np[c, s] = blin - Wm_s - Wx_s - n_g * Ws_s
    wlin_np = np.zeros((64, 6), np.float32)
    wlin_np[:, 0:2] = Wlin[0:64]
    wlin_np[:, 2:4] = Wlin[64:128]
    wlin_np[:, 4:6] = Wlin[128:192]

    xpT_np = np.zeros((NCORES, 8, NLOC), np.float16)
    for c in range(NCORES):
        mc = core_n == c
        xpT_np[c, 0:IN, loc[mc]] = x[mc].astype(np.float16)
        xpT_np[c, IN, loc[mc]] = 1.0

    cfg = dict(G_PAD=G_PAD, K1=tuple(int(v) for v in K1),
               K2A=tuple(int(v) for v in K2A),
               K2B=tuple(int(v) for v in K2B),
               UPAD=tuple(int(v) for v in UPAD))

    in_maps = []
    for c in range(NCORES):
        in_maps.append(dict(
            xpT=xpT_np[c], xtab=xtab_np[c], xidx=xidx_np[c],
            xr1i=xr1i_np[c], ds1=ds1_np[c], xleAi=xleAi_np[c],
            xleBi=xleBi_np[c], xr2i=xr2i_np[c], ds2=ds2_np[c],
            wlr1=wlr1_np, att1b=att1b_np, bias1b=bias1b_np,
            w2cat=w2cat_np, blr2b=blr2b_np, att2b=att2b_np,
            bias2b=bias2b_np, iota16=iota_np, mk8=mk8_np[c],
            bgp=bgp_np[c], rcntT=rcntT_np[c], wlin=wlin_np,
            blinb=blinb_np[c],
        ))
    return cfg, in_maps


def kernel(**inputs):
    global _last
    cfg, in_maps = _prep(inputs)
    key = (cfg["G_PAD"], cfg["K1"], cfg["K2A"], cfg["K2B"], cfg["UPAD"])
    if key not in _cache:
        _cache[key] = _build(cfg)
    nc = _cache[key]
    _last = (nc, in_maps)
    res = run_bass_kernel_spmd(nc, in_maps, list(range(NCORES)), trace=False)
    out = np.concatenate([res.results[c]["pooled"] for c in range(NCORES)],
                         axis=0)
    return out.astype(np.float32)


def _rerun():
    nc, in_maps = _last
    return run_bass_kernel_spmd(nc, in_maps, list(range(NCORES)),
                                trace=False)

